# revision 22
# baseline (speedup 1.0000x reference)
"""CompoundHeadAttention TRN2 kernel.

Full-input contract: kernel(**inputs) takes the unsharded tensors from
setup_inputs() and returns the full [1, 2048, 2048] float32 output.

Sharding (8 cores, tensor-parallel over the HK=8 kv heads):
  core h owns kv head h: its Wq/Wk/Wv column slice, its WG[h]/bG[h], and
  Wfc row-slice [h*256:(h+1)*256, :].  Each core computes its head's
  attention + its partial FC output [2048, 2048] (fp16); the host sums
  the 8 partials and adds bfc (the "all-reduce" of the row-sharded FC).

Device-side structure per core (N=2048, E=2048, D=64, G=4):
  - per 512-token window w: project Q/K/V (fp16 matmuls, fp32 psum),
    G-transform Q into qg01/qg23 (f32r), V transposed to [s, 64|1] bf16.
  - attention per window j over s-chunks of 128, software-pipelined:
    ST(k) matmuls -> exp on ACT (pt bf16) -> mask-mul on DVE (static
    [128,128] triangle mask) -> PV(k-1) matmuls.  PV lags one chunk so
    the PE never waits on the ACT engine.
  - normalize: reciprocal_approx_fast (DVE) + gpsimd partition
    broadcast + DVE mul.
  - FC: hid^T @ Wfc rows into psum, DVE-drained to fp16 staging, DMA'd
    out from the ACT queue.
  Input DMAs are batched 4 e-chunks at a time and prefetched; junk
  matmuls keep the PE busy during the DMA-bound ramp so the HAM clock
  gate reaches full speed early.
"""

import os
import sys

import numpy as np

if "/opt/trn_rl_repo" not in sys.path and os.path.isdir("/opt/trn_rl_repo"):
    sys.path.insert(0, "/opt/trn_rl_repo")

import concourse.bass as bass  # noqa: E402
import concourse.mybir as mybir  # noqa: E402
import concourse.tile as tile  # noqa: E402
from concourse import bacc  # noqa: E402
from concourse import bass_utils  # noqa: E402

F32 = mybir.dt.float32
F32R = mybir.dt.float32r
F16 = mybir.dt.float16
BF16 = mybir.dt.bfloat16
AF = mybir.ActivationFunctionType

N = 2048
E = 2048
HK = 8
D = 64
G = 4
NB = 4        # 512-wide n-windows


def build_program():
    nc = bacc.Bacc("TRN2", target_bir_lowering=False, debug=False,
                   enable_asserts=False)

    # ---- DRAM I/O ----
    # inputs pre-chunked host-side: [16 e-chunks, 128, N]
    qT = nc.dram_tensor("qT", [16, 128, N], F16, kind="ExternalInput").ap()
    kT = nc.dram_tensor("kT", [16, 128, N], F16, kind="ExternalInput").ap()
    vT = nc.dram_tensor("vT", [16, 128, N], F16, kind="ExternalInput").ap()
    # weight chunk layout: [128, 16*M] — e-chunk ec occupies cols [M*ec, M*ec+M)
    wq = nc.dram_tensor("wq", [128, 16 * 128], F16, kind="ExternalInput").ap()
    wk = nc.dram_tensor("wk", [128, 16 * 128], F16, kind="ExternalInput").ap()
    wv = nc.dram_tensor("wv", [128, 16 * 64], F16, kind="ExternalInput").ap()
    bq2 = nc.dram_tensor("bq2", [128, 1], F32, kind="ExternalInput").ap()
    bk2 = nc.dram_tensor("bk2", [128, 1], F32, kind="ExternalInput").ap()
    bvv = nc.dram_tensor("bvv", [64, 1], F32, kind="ExternalInput").ap()
    wg = nc.dram_tensor("wg", [128, 256], F32R, kind="ExternalInput").ap()
    bg01 = nc.dram_tensor("bg01", [128, 1], F32, kind="ExternalInput").ap()
    bg23 = nc.dram_tensor("bg23", [128, 1], F32, kind="ExternalInput").ap()
    wfc = nc.dram_tensor("wfc", [256, E], F32R, kind="ExternalInput").ap()
    ident = nc.dram_tensor("ident", [64, 64], F32, kind="ExternalInput").ap()
    # causal mask band [128, 2*128] bf16: [M | M], M[p,c] = (c >= p)
    mask2 = nc.dram_tensor("mask2", [128, 256], BF16, kind="ExternalInput").ap()
    # output row-chunked: [16, 128, E] fp16
    out = nc.dram_tensor("out", [16, 128, E], F16, kind="ExternalOutput").ap()

    with tile.TileContext(nc) as tc:
        build_tile_kernel(tc, qT=qT, kT=kT, vT=vT, wq=wq, wk=wk, wv=wv,
                          bq2=bq2, bk2=bk2, bvv=bvv, wg=wg, bg01=bg01,
                          bg23=bg23, wfc=wfc, ident=ident, mask2=mask2,
                          out=out)
    nc.compile()
    return nc


def build_tile_kernel(tc, *, qT, kT, vT, wq, wk, wv, bq2, bk2, bvv, wg,
                      bg01, bg23, wfc, ident, mask2, out):
    nc = tc.nc

    import contextlib
    ctx = contextlib.ExitStack()
    ctx.__enter__()
    cp = ctx.enter_context(tc.tile_pool(name="persist", bufs=1))

    def ptile(shape, dtype, name):
        return cp.tile(shape, dtype, tag=name, name=name)

    # ---- persistent constants in SBUF ----
    wq_sb = ptile([128, 16 * 128], F16, "wq_sb")
    wk_sb = ptile([128, 16 * 128], F16, "wk_sb")
    wv_sb = ptile([128, 16 * 64], F16, "wv_sb")
    wg_sb = ptile([128, 256], F32R, "wg_sb")
    wfc0_sb = ptile([128, E], F32R, "wfc0_sb")
    wfc1_sb = ptile([128, E], F32R, "wfc1_sb")
    id_sb = ptile([64, 64], F32, "id_sb")
    mask_sb = ptile([128, 256], BF16, "mask_sb")
    bq_sb = ptile([128, 1], F32, "bq_sb")
    bk_sb = ptile([128, 1], F32, "bk_sb")
    bv_sb = ptile([64, 1], F32, "bv_sb")
    bg01_sb = ptile([128, 1], F32, "bg01_sb")
    bg23_sb = ptile([128, 1], F32, "bg23_sb")
    ones_sb = ptile([128, 1], F32, "ones_sb")
    nc.vector.memset(ones_sb[:], 1.0)
    # -50*ln(2): cancels the 2^-50 pre-scale fed into ln (see do_norm)
    nl2_sb = ptile([128, 1], F32, "nl2_sb")
    nc.vector.memset(nl2_sb[:], -34.657359028)

    # only wq is needed before the first q rows can be consumed; the
    # remaining consts trickle in between the first loads (emit_consts).
    nc.sync.dma_start(wq_sb[:], wq[:])

    # per-window persistent activations
    kt_w = [ptile([128, 512], F32R, f"kt{j}") for j in range(NB)]
    vo_w = [ptile([128, 4 * 65], BF16, f"vo{j}") for j in range(NB)]
    qg01_w = [ptile([128, 512], F32R, f"qg01_{j}") for j in range(NB)]
    qg23_w = [ptile([128, 512], F32R, f"qg23_{j}") for j in range(NB)]
    hid01_w = [ptile([128, 512], F32R, f"hid01_{j}") for j in range(NB)]
    hid23_w = [ptile([128, 512], F32R, f"hid23_{j}") for j in range(NB)]

    mask3 = mask_sb[:].rearrange("p (h c) -> p h c", c=128)

    with ctx:
        in_pool = ctx.enter_context(tc.tile_pool(name="in_pool", bufs=7))
        qt_pool = ctx.enter_context(tc.tile_pool(name="qt_pool", bufs=2))
        vt_pool = ctx.enter_context(tc.tile_pool(name="vt_pool", bufs=2))
        pt_pool = ctx.enter_context(tc.tile_pool(name="pt_pool", bufs=3))
        rec_pool = ctx.enter_context(tc.tile_pool(name="rec_pool", bufs=2))
        fco_pool = ctx.enter_context(tc.tile_pool(name="fco_pool", bufs=2))
        misc_ps = ctx.enter_context(
            tc.tile_pool(name="misc_ps", bufs=2, space="PSUM"))
        st_ps = ctx.enter_context(
            tc.tile_pool(name="st_ps", bufs=2, space="PSUM"))
        pv_ps = ctx.enter_context(
            tc.tile_pool(name="pv_ps", bufs=2, space="PSUM"))

        in_tiles = {}  # (w, tensor_idx, block) -> sbuf tile

        SRC = (qT, kT, vT)

        def emit_loads(w):
            """issue the 12 input block-DMAs for window w (4KB/part each)"""
            for ti in range(3):
                for c in range(4):
                    t = in_pool.tile([128, 4, 512], F16, tag="in",
                                     name=f"in_{w}_{ti}_{c}")
                    in_tiles[(w, ti, c)] = t
                    nc.sync.dma_start(
                        t[:],
                        SRC[ti][bass.ds(4 * c, 4), :, bass.ds(512 * w, 512)]
                        .rearrange("c p n -> p c n"))
                    yield

        def emit_proj(w):
            """projections + G for window w; coarse bursts (1 tensor/yield)"""
            # Q
            q_ps = misc_ps.tile([128, 512], F32, tag="mm", name="q_ps")
            for c in range(4):
                t = in_tiles.pop((w, 0, c))
                for e in range(4):
                    ec = 4 * c + e
                    nc.tensor.matmul(q_ps[:], wq_sb[:, bass.ts(ec, 128)],
                                     t[:, e, :], start=(ec == 0),
                                     stop=(ec == 15))
            qt = qt_pool.tile([128, 512], F32R, tag="qt", name="qt")
            nc.scalar.activation(qt[:], q_ps[:], AF.Identity, bias=bq_sb[:])
            yield
            # G transform
            g01_ps = misc_ps.tile([128, 512], F32, tag="mm", name="g01_ps")
            g23_ps = misc_ps.tile([128, 512], F32, tag="mm", name="g23_ps")
            nc.tensor.matmul(g01_ps[:], wg_sb[0:64, 0:128], qt[0:64, :],
                             start=True, stop=True)
            nc.tensor.matmul(g23_ps[:], wg_sb[64:128, 128:256], qt[64:128, :],
                             start=True, stop=True)
            nc.scalar.activation(qg01_w[w][:], g01_ps[:], AF.Identity,
                                 bias=bg01_sb[:])
            nc.scalar.activation(qg23_w[w][:], g23_ps[:], AF.Identity,
                                 bias=bg23_sb[:])
            yield
            # K
            k_ps = misc_ps.tile([128, 512], F32, tag="mm", name="k_ps")
            for c in range(4):
                t = in_tiles.pop((w, 1, c))
                for e in range(4):
                    ec = 4 * c + e
                    nc.tensor.matmul(k_ps[:], wk_sb[:, bass.ts(ec, 128)],
                                     t[:, e, :], start=(ec == 0),
                                     stop=(ec == 15))
            nc.scalar.activation(kt_w[w][:], k_ps[:], AF.Identity,
                                 bias=bk_sb[:])
            yield
            # V (VT then PE-transpose to V layout, + ones column)
            v_ps = misc_ps.tile([64, 512], F32, tag="mm", name="v_ps")
            for c in range(4):
                t = in_tiles.pop((w, 2, c))
                for e in range(4):
                    ec = 4 * c + e
                    nc.tensor.matmul(v_ps[:], wv_sb[:, bass.ts(ec, 64)],
                                     t[:, e, :], start=(ec == 0),
                                     stop=(ec == 15))
            vt_sb = vt_pool.tile([64, 512], F32, tag="vt", name="vt_sb")
            nc.scalar.activation(vt_sb[:], v_ps[:], AF.Identity,
                                 bias=bv_sb[:])
            yield
            tr_ps = misc_ps.tile([128, 256], F32, tag="mm", name="tr_ps")
            for t4 in range(4):
                nc.tensor.transpose(tr_ps[:, bass.ts(t4, 64)],
                                    vt_sb[:, bass.ts(t4, 128)], id_sb[:])
            for t4 in range(4):
                nc.vector.tensor_copy(vo_w[w][:, t4 * 65:t4 * 65 + 64],
                                      tr_ps[:, bass.ts(t4, 64)])
                nc.vector.tensor_copy(
                    vo_w[w][:, t4 * 65 + 64:t4 * 65 + 65], ones_sb[:])
            yield

        def emit_attn(j):
            klast = 4 * j + 3
            jobs = [(pair, k) for pair in (0, 1) for k in range(klast + 1)]
            pv_tiles = {}
            pend = None  # (pair, k, pt, pv_off)

            def do_st(pair, k):
                i = k - 4 * j
                st_off = 0 if i < 0 else min(128 * i, 256)
                pv_off = 0 if i < 0 else 128 * i
                kt_c = kt_w[k // 4]
                ks = bass.ts(k % 4, 128)
                qg = qg01_w[j] if pair == 0 else qg23_w[j]
                st = st_ps.tile([128, 1024], F32, tag="st", name="st")
                nc.tensor.matmul(st[:, st_off:512], kt_c[0:64, ks],
                                 qg[0:64, st_off:512], start=True, stop=True)
                nc.tensor.matmul(st[:, 512 + st_off:1024], kt_c[64:128, ks],
                                 qg[64:128, st_off:512], start=True, stop=True)
                pt = pt_pool.tile([128, 1024], BF16, tag="pt", name="pt")
                st3 = st[:].rearrange("p (h c) -> p h c", c=512)
                pt3 = pt[:].rearrange("p (h c) -> p h c", c=512)
                # unshifted exp(8*st): the HW exp table has an absolute
                # error floor, so shifting all logits down corrupts rows
                # whose terms are all tiny; fp32/bf16 absorb e^75 fine.
                nc.scalar.activation(pt3[:, :, pv_off:512],
                                     st3[:, :, pv_off:512], AF.Exp,
                                     scale=8.0)
                if i >= 0:
                    # zero the upper triangle of the diagonal band: keep
                    # where local col c >= partition p
                    band = pt3[:, :, pv_off:pv_off + 128]
                    nc.gpsimd.affine_select(
                        out=band, in_=band,
                        compare_op=mybir.AluOpType.is_ge,
                        fill=0.0, base=0,
                        pattern=[[0, 2], [1, 128]],
                        channel_multiplier=-1)
                return pt, pv_off

            def do_pv(pair, k, pt, pv_off):
                if k == 0:
                    pv_tiles[pair] = (
                        pv_ps.tile([65, 512], F32, tag="pv", name="pv_a"),
                        pv_ps.tile([65, 512], F32, tag="pv", name="pv_b"))
                pv_a, pv_b = pv_tiles[pair]
                vo_c = vo_w[k // 4]
                vsl = vo_c[:, (k % 4) * 65:(k % 4) * 65 + 65]
                nc.tensor.matmul(pv_a[:, pv_off:512], vsl,
                                 pt[:, pv_off:512],
                                 start=(k == 0), stop=(k == klast))
                nc.tensor.matmul(pv_b[:, pv_off:512], vsl,
                                 pt[:, 512 + pv_off:1024],
                                 start=(k == 0), stop=(k == klast))

            def do_norm(pair):
                hid = hid01_w[j] if pair == 0 else hid23_w[j]
                pv_a, pv_b = pv_tiles.pop(pair)
                # drain pv psum to SBUF immediately (frees the banks for the
                # next pair's accumulation without waiting on the ACT queue)
                pvc = []
                for pv in (pv_a, pv_b):
                    c = rec_pool.tile([65, 512], F32, tag="pvc", name="pvc")
                    nc.vector.tensor_copy(c[:], pv[:])
                    pvc.append(c)
                for half, pv in ((0, pvc[0]), (1, pvc[1])):
                    # 1/den = exp(-ln(den*2^-50) - 50*ln2) on ACT: ln+exp
                    # share one activation table with the softmax exp (no
                    # swaps); the 2^-50 pre-scale keeps ln's argument
                    # inside its [-2^64, 2^64] domain (den reaches ~8e32)
                    lnd = rec_pool.tile([1, 512], F32, tag="lnd", name="lnd")
                    nc.scalar.activation(lnd[:], pv[64:65, :], AF.Ln,
                                         scale=2.0 ** -50)
                    rec = rec_pool.tile([1, 512], F32, tag="rec", name="rec")
                    nc.scalar.activation(rec[:], lnd[:], AF.Exp, scale=-1.0,
                                         bias=nl2_sb[0:1, :])
                    recr = rec_pool.tile([64, 512], F32, tag="recr",
                                         name="recr")
                    nc.gpsimd.partition_broadcast(recr[:], rec[:])
                    nc.vector.tensor_mul(hid[half * 64:half * 64 + 64, :],
                                         pv[0:64, :], recr[:])

            for (pair, k) in jobs:
                cur = do_st(pair, k)
                if pend is not None:
                    do_pv(*pend)
                    if pend[1] == klast:  # pair-01 accumulation finished
                        do_norm(pend[0])
                        yield  # extra turn: fc/proj filler covers norm drain
                pend = (pair, k) + cur
                yield
            do_pv(*pend)
            do_norm(pend[0])

        def emit_fc(j, tail=False):
            nd = 0
            for mm2 in range(2):
                stage = fco_pool.tile([128, 2, 2048], F16, tag="fco",
                                      name="stage")
                for m2 in range(2):
                    m = 2 * mm2 + m2
                    msl = bass.ts(m, 128)
                    for eo in range(4):
                        fc_ps = misc_ps.tile([128, 512], F32, tag="mm",
                                             name="fc_ps")
                        nc.tensor.matmul(fc_ps[:], hid01_w[j][:, msl],
                                         wfc0_sb[:, bass.ts(eo, 512)],
                                         start=True, stop=False)
                        nc.tensor.matmul(fc_ps[:], hid23_w[j][:, msl],
                                         wfc1_sb[:, bass.ts(eo, 512)],
                                         start=False, stop=True)
                        dst = stage[:, m2, bass.ts(eo, 512)]
                        if tail and nd % 2 == 1:
                            # at the tail ACT is idle: alternate drains so
                            # the psum pool is never drain-paced
                            nc.scalar.activation(dst, fc_ps[:], AF.Copy)
                        else:
                            nc.vector.tensor_copy(dst, fc_ps[:])
                        nd += 1
                        yield
                nc.scalar.dma_start(
                    out[bass.ds(4 * j + 2 * mm2, 2), :, :]
                    .rearrange("c p n -> p c n"),
                    stage[:])

        def emit_consts():
            # ordered by first use: Q-bias, G, K, V, transpose, mask, FC
            for dst, srcap in ((bq_sb, bq2), (wg_sb, wg), (bg01_sb, bg01),
                               (bg23_sb, bg23), (wk_sb, wk), (bk_sb, bk2),
                               (wv_sb, wv), (bv_sb, bvv), (id_sb, ident),
                               (mask_sb, mask2)):
                nc.sync.dma_start(dst[:], srcap[:])
                yield
            nc.sync.dma_start(wfc0_sb[:], wfc[0:128, :])
            yield
            nc.sync.dma_start(wfc1_sb[:], wfc[128:256, :])
            yield

        def emit_warm(n):
            # junk matmuls: keep the PE busy during the DMA-bound ramp so
            # the HAM clock gate reaches full speed before real work peaks
            for _ in range(n):
                jt = pv_ps.tile([65, 512], F32, tag="pv", name="warm")
                nc.tensor.matmul(jt[:], wq_sb[:, 0:65], wq_sb[:, 0:512],
                                 start=True, stop=True)
                yield

        from itertools import chain as ichain

        def drain(g):
            for _ in g:
                pass

        def rr(pairs):
            """round-robin emission: [(generator, steps_per_turn)]"""
            live = [[g, w] for g, w in pairs]
            while live:
                for gw in list(live):
                    g, w = gw
                    try:
                        for _ in range(w):
                            next(g)
                    except StopIteration:
                        live.remove(gw)

        # Phase A: window-0/1 loads + consts + proj(0), PE kept warm
        rr([(ichain(emit_loads(0), emit_loads(1)), 4),
            (emit_consts(), 3), (emit_proj(0), 1), (emit_warm(60), 4)])
        # Phase B: attn(0) with proj(1) + loads(2)
        rr([(emit_attn(0), 1), (emit_loads(2), 2), (emit_proj(1), 1)])
        # Phase C: attn(1) + proj(2) + loads(3) + fc(0)
        rr([(emit_attn(1), 1), (emit_loads(3), 2), (emit_proj(2), 1),
            (emit_fc(0), 1)])
        # Phase D: attn(2) + proj(3) + fc(1)
        rr([(emit_attn(2), 1), (emit_proj(3), 1), (emit_fc(1), 1)])
        # Phase E: attn(3) + fc(2)
        rr([(emit_attn(3), 1), (emit_fc(2), 1)])
        drain(emit_fc(3, tail=True))


def shard_inputs(inputs):
    """full inputs -> list of 8 per-core in_maps (numpy, device layouts)"""
    f16 = np.float16
    f32 = np.float32
    bf16 = None
    import ml_dtypes
    bf16 = ml_dtypes.bfloat16
    q = np.asarray(inputs["q"], f32)[0]
    k = np.asarray(inputs["k"], f32)[0]
    v = np.asarray(inputs["v"], f32)[0]
    Wq = np.asarray(inputs["Wq"], f32)
    Wk = np.asarray(inputs["Wk"], f32)
    Wv = np.asarray(inputs["Wv"], f32)
    bq = np.asarray(inputs["bq"], f32)
    bk = np.asarray(inputs["bk"], f32)
    bv = np.asarray(inputs["bv"], f32)
    WG = np.asarray(inputs["WG"], f32)
    bG = np.asarray(inputs["bG"], f32)
    Wfc = np.asarray(inputs["Wfc"], f32)

    qT = np.ascontiguousarray(q.T.astype(f16)).reshape(16, 128, N)
    kT = np.ascontiguousarray(k.T.astype(f16)).reshape(16, 128, N)
    vT = np.ascontiguousarray(v.T.astype(f16)).reshape(16, 128, N)
    ident = np.eye(64, dtype=f32)
    # mask band M[p, c] = 1 if c >= p else 0, duplicated: [M | M]
    M = (np.arange(128)[None, :] >= np.arange(128)[:, None]).astype(bf16)
    mask2 = np.concatenate([M, M], axis=1).copy()

    def chunked(w):
        # [E, M] -> [128, 16*M]: e-chunk ec at cols [M*ec, M*ec+M)
        M_ = w.shape[1]
        return np.ascontiguousarray(
            w.reshape(16, 128, M_).transpose(1, 0, 2).reshape(128, 16 * M_))

    maps = []
    for h in range(HK):
        sl = slice(h * D, (h + 1) * D)
        wq_h = Wq[:, sl]
        wk_h = Wk[:, sl]
        wv_h = Wv[:, sl]
        m = {
            "qT": qT, "kT": kT, "vT": vT,
            "wq": chunked(np.concatenate([wq_h, wq_h], 1)).astype(f16),
            "wk": chunked(np.concatenate([wk_h, wk_h], 1)).astype(f16),
            "wv": chunked(wv_h).astype(f16),
            "bq2": np.concatenate([bq[sl], bq[sl]]).reshape(128, 1).copy(),
            "bk2": np.concatenate([bk[sl], bk[sl]]).reshape(128, 1).copy(),
            "bvv": bv[sl].reshape(64, 1).copy(),
            "wg": np.concatenate([WG[h], WG[h]], 0).copy(),  # [128, 256]
            "bg01": bG[h, 0:128].reshape(128, 1).copy(),
            "bg23": bG[h, 128:256].reshape(128, 1).copy(),
            "wfc": Wfc[h * 256:(h + 1) * 256, :].copy(),
            "ident": ident,
            "mask2": mask2,
        }
        maps.append(m)
    return maps


_compiled = None
last_results = None


def get_compiled():
    global _compiled
    if _compiled is None:
        _compiled = build_program()
    return _compiled


def kernel(**inputs):
    global last_results
    nc = get_compiled()
    in_maps = shard_inputs(inputs)
    last_results = bass_utils.run_bass_kernel_spmd(
        nc, in_maps, core_ids=list(range(8)))
    bfc = np.asarray(inputs["bfc"], np.float32)
    acc = np.zeros((N, E), np.float32)
    for res in last_results.results:
        acc += res["out"].reshape(N, E).astype(np.float32)
    full = acc + bfc[None, :]
    return full.reshape(1, N, E)


# revision 23
# speedup vs baseline: 1.0088x; 1.0088x over previous
"""CompoundHeadAttention TRN2 kernel.

Full-input contract: kernel(**inputs) takes the unsharded tensors from
setup_inputs() and returns the full [1, 2048, 2048] float32 output.

Sharding (8 cores, tensor-parallel over the HK=8 kv heads):
  core h owns kv head h: its Wq/Wk/Wv column slice, its WG[h]/bG[h], and
  Wfc row-slice [h*256:(h+1)*256, :].  Each core computes its head's
  attention + its partial FC output [2048, 2048] (fp16); the host sums
  the 8 partials and adds bfc (the "all-reduce" of the row-sharded FC).

Device-side structure per core (N=2048, E=2048, D=64, G=4):
  - per 512-token window w: project Q/K/V (fp16 matmuls, fp32 psum),
    G-transform Q into qg01/qg23 (f32r), V transposed to [s, 64|1] bf16.
  - attention per window j over s-chunks of 128, software-pipelined:
    ST(k) matmuls -> exp on ACT (pt bf16) -> mask-mul on DVE (static
    [128,128] triangle mask) -> PV(k-1) matmuls.  PV lags one chunk so
    the PE never waits on the ACT engine.
  - normalize: reciprocal_approx_fast (DVE) + gpsimd partition
    broadcast + DVE mul.
  - FC: hid^T @ Wfc rows into psum, DVE-drained to fp16 staging, DMA'd
    out from the ACT queue.
  Input DMAs are batched 4 e-chunks at a time and prefetched; junk
  matmuls keep the PE busy during the DMA-bound ramp so the HAM clock
  gate reaches full speed early.
"""

import os
import sys

import numpy as np

if "/opt/trn_rl_repo" not in sys.path and os.path.isdir("/opt/trn_rl_repo"):
    sys.path.insert(0, "/opt/trn_rl_repo")

import concourse.bass as bass  # noqa: E402
import concourse.mybir as mybir  # noqa: E402
import concourse.tile as tile  # noqa: E402
from concourse import bacc  # noqa: E402
from concourse import bass_utils  # noqa: E402

F32 = mybir.dt.float32
F32R = mybir.dt.float32r
F16 = mybir.dt.float16
BF16 = mybir.dt.bfloat16
AF = mybir.ActivationFunctionType

N = 2048
E = 2048
HK = 8
D = 64
G = 4
NB = 4        # 512-wide n-windows


def build_program():
    nc = bacc.Bacc("TRN2", target_bir_lowering=False, debug=False,
                   enable_asserts=False)

    # ---- DRAM I/O ----
    # inputs pre-chunked host-side: [16 e-chunks, 128, N]
    qT = nc.dram_tensor("qT", [16, 128, N], F16, kind="ExternalInput").ap()
    kT = nc.dram_tensor("kT", [16, 128, N], F16, kind="ExternalInput").ap()
    vT = nc.dram_tensor("vT", [16, 128, N], F16, kind="ExternalInput").ap()
    # weight chunk layout: [128, 16*M] — e-chunk ec occupies cols [M*ec, M*ec+M)
    wq = nc.dram_tensor("wq", [128, 16 * 128], F16, kind="ExternalInput").ap()
    wk = nc.dram_tensor("wk", [128, 16 * 128], F16, kind="ExternalInput").ap()
    wv = nc.dram_tensor("wv", [128, 16 * 64], F16, kind="ExternalInput").ap()
    bq2 = nc.dram_tensor("bq2", [128, 1], F32, kind="ExternalInput").ap()
    bk2 = nc.dram_tensor("bk2", [128, 1], F32, kind="ExternalInput").ap()
    bvv = nc.dram_tensor("bvv", [64, 1], F32, kind="ExternalInput").ap()
    wg = nc.dram_tensor("wg", [128, 256], F32R, kind="ExternalInput").ap()
    bg01 = nc.dram_tensor("bg01", [128, 1], F32, kind="ExternalInput").ap()
    bg23 = nc.dram_tensor("bg23", [128, 1], F32, kind="ExternalInput").ap()
    wfc = nc.dram_tensor("wfc", [256, E], F32R, kind="ExternalInput").ap()
    ident = nc.dram_tensor("ident", [64, 64], F32, kind="ExternalInput").ap()
    # causal mask band [128, 2*128] bf16: [M | M], M[p,c] = (c >= p)
    mask2 = nc.dram_tensor("mask2", [128, 256], BF16, kind="ExternalInput").ap()
    # output row-chunked: [16, 128, E] fp16
    out = nc.dram_tensor("out", [16, 128, E], F16, kind="ExternalOutput").ap()

    with tile.TileContext(nc) as tc:
        build_tile_kernel(tc, qT=qT, kT=kT, vT=vT, wq=wq, wk=wk, wv=wv,
                          bq2=bq2, bk2=bk2, bvv=bvv, wg=wg, bg01=bg01,
                          bg23=bg23, wfc=wfc, ident=ident, mask2=mask2,
                          out=out)
    nc.compile()
    return nc


def build_tile_kernel(tc, *, qT, kT, vT, wq, wk, wv, bq2, bk2, bvv, wg,
                      bg01, bg23, wfc, ident, mask2, out):
    nc = tc.nc

    import contextlib
    ctx = contextlib.ExitStack()
    ctx.__enter__()
    cp = ctx.enter_context(tc.tile_pool(name="persist", bufs=1))

    def ptile(shape, dtype, name):
        return cp.tile(shape, dtype, tag=name, name=name)

    # ---- persistent constants in SBUF ----
    wq_sb = ptile([128, 16 * 128], F16, "wq_sb")
    wk_sb = ptile([128, 16 * 128], F16, "wk_sb")
    wv_sb = ptile([128, 16 * 64], F16, "wv_sb")
    wg_sb = ptile([128, 256], F32R, "wg_sb")
    wfc0_sb = ptile([128, E], F32R, "wfc0_sb")
    wfc1_sb = ptile([128, E], F32R, "wfc1_sb")
    id_sb = ptile([64, 64], F32, "id_sb")
    mask_sb = ptile([128, 256], BF16, "mask_sb")
    bq_sb = ptile([128, 1], F32, "bq_sb")
    bk_sb = ptile([128, 1], F32, "bk_sb")
    bv_sb = ptile([64, 1], F32, "bv_sb")
    bg01_sb = ptile([128, 1], F32, "bg01_sb")
    bg23_sb = ptile([128, 1], F32, "bg23_sb")
    ones_sb = ptile([128, 1], F32, "ones_sb")
    nc.vector.memset(ones_sb[:], 1.0)
    # -50*ln(2): cancels the 2^-50 pre-scale fed into ln (see do_norm)
    nl2_sb = ptile([128, 1], F32, "nl2_sb")
    nc.vector.memset(nl2_sb[:], -34.657359028)

    # only wq is needed before the first q rows can be consumed; the
    # remaining consts trickle in between the first loads (emit_consts).
    nc.sync.dma_start(wq_sb[:], wq[:])

    # per-window persistent activations
    kt_w = [ptile([128, 512], F32R, f"kt{j}") for j in range(NB)]
    vo_w = [ptile([128, 4 * 65], BF16, f"vo{j}") for j in range(NB)]
    qg01_w = [ptile([128, 512], F32R, f"qg01_{j}") for j in range(NB)]
    qg23_w = [ptile([128, 512], F32R, f"qg23_{j}") for j in range(NB)]
    hid01_w = [ptile([128, 512], F32R, f"hid01_{j}") for j in range(NB)]
    hid23_w = [ptile([128, 512], F32R, f"hid23_{j}") for j in range(NB)]

    mask3 = mask_sb[:].rearrange("p (h c) -> p h c", c=128)

    with ctx:
        in_pool = ctx.enter_context(tc.tile_pool(name="in_pool", bufs=7))
        qt_pool = ctx.enter_context(tc.tile_pool(name="qt_pool", bufs=2))
        vt_pool = ctx.enter_context(tc.tile_pool(name="vt_pool", bufs=2))
        pt_pool = ctx.enter_context(tc.tile_pool(name="pt_pool", bufs=3))
        rec_pool = ctx.enter_context(tc.tile_pool(name="rec_pool", bufs=2))
        fco_pool = ctx.enter_context(tc.tile_pool(name="fco_pool", bufs=2))
        misc_ps = ctx.enter_context(
            tc.tile_pool(name="misc_ps", bufs=2, space="PSUM"))
        st_ps = ctx.enter_context(
            tc.tile_pool(name="st_ps", bufs=2, space="PSUM"))
        pv_ps = ctx.enter_context(
            tc.tile_pool(name="pv_ps", bufs=2, space="PSUM"))

        in_tiles = {}  # (w, tensor_idx, block) -> sbuf tile

        SRC = (qT, kT, vT)

        def emit_loads(w):
            """issue the 12 input block-DMAs for window w (4KB/part each)"""
            for ti in range(3):
                for c in range(4):
                    t = in_pool.tile([128, 4, 512], F16, tag="in",
                                     name=f"in_{w}_{ti}_{c}")
                    in_tiles[(w, ti, c)] = t
                    nc.sync.dma_start(
                        t[:],
                        SRC[ti][bass.ds(4 * c, 4), :, bass.ds(512 * w, 512)]
                        .rearrange("c p n -> p c n"))
                    yield

        def emit_proj(w):
            """projections + G for window w; coarse bursts (1 tensor/yield)"""
            # Q
            q_ps = misc_ps.tile([128, 512], F32, tag="mm", name="q_ps")
            for c in range(4):
                t = in_tiles.pop((w, 0, c))
                for e in range(4):
                    ec = 4 * c + e
                    nc.tensor.matmul(q_ps[:], wq_sb[:, bass.ts(ec, 128)],
                                     t[:, e, :], start=(ec == 0),
                                     stop=(ec == 15))
            qt = qt_pool.tile([128, 512], F32R, tag="qt", name="qt")
            nc.scalar.activation(qt[:], q_ps[:], AF.Identity, bias=bq_sb[:])
            yield
            # G transform
            g01_ps = misc_ps.tile([128, 512], F32, tag="mm", name="g01_ps")
            g23_ps = misc_ps.tile([128, 512], F32, tag="mm", name="g23_ps")
            nc.tensor.matmul(g01_ps[:], wg_sb[0:64, 0:128], qt[0:64, :],
                             start=True, stop=True)
            nc.tensor.matmul(g23_ps[:], wg_sb[64:128, 128:256], qt[64:128, :],
                             start=True, stop=True)
            nc.scalar.activation(qg01_w[w][:], g01_ps[:], AF.Identity,
                                 bias=bg01_sb[:])
            nc.scalar.activation(qg23_w[w][:], g23_ps[:], AF.Identity,
                                 bias=bg23_sb[:])
            yield
            # K
            k_ps = misc_ps.tile([128, 512], F32, tag="mm", name="k_ps")
            for c in range(4):
                t = in_tiles.pop((w, 1, c))
                for e in range(4):
                    ec = 4 * c + e
                    nc.tensor.matmul(k_ps[:], wk_sb[:, bass.ts(ec, 128)],
                                     t[:, e, :], start=(ec == 0),
                                     stop=(ec == 15))
            nc.scalar.activation(kt_w[w][:], k_ps[:], AF.Identity,
                                 bias=bk_sb[:])
            yield
            # V (VT then PE-transpose to V layout, + ones column)
            v_ps = misc_ps.tile([64, 512], F32, tag="mm", name="v_ps")
            for c in range(4):
                t = in_tiles.pop((w, 2, c))
                for e in range(4):
                    ec = 4 * c + e
                    nc.tensor.matmul(v_ps[:], wv_sb[:, bass.ts(ec, 64)],
                                     t[:, e, :], start=(ec == 0),
                                     stop=(ec == 15))
            vt_sb = vt_pool.tile([64, 512], F32, tag="vt", name="vt_sb")
            nc.scalar.activation(vt_sb[:], v_ps[:], AF.Identity,
                                 bias=bv_sb[:])
            yield
            tr_ps = misc_ps.tile([128, 256], F32, tag="mm", name="tr_ps")
            for t4 in range(4):
                nc.tensor.transpose(tr_ps[:, bass.ts(t4, 64)],
                                    vt_sb[:, bass.ts(t4, 128)], id_sb[:])
            for t4 in range(4):
                nc.vector.tensor_copy(vo_w[w][:, t4 * 65:t4 * 65 + 64],
                                      tr_ps[:, bass.ts(t4, 64)])
                nc.vector.tensor_copy(
                    vo_w[w][:, t4 * 65 + 64:t4 * 65 + 65], ones_sb[:])
            yield

        def emit_attn(j):
            klast = 4 * j + 3
            jobs = [(pair, k) for pair in (0, 1) for k in range(klast + 1)]
            pv_tiles = {}
            pend = None  # (pair, k, pt, pv_off)

            def do_st(pair, k):
                i = k - 4 * j
                st_off = 0 if i < 0 else min(128 * i, 256)
                pv_off = 0 if i < 0 else 128 * i
                kt_c = kt_w[k // 4]
                ks = bass.ts(k % 4, 128)
                qg = qg01_w[j] if pair == 0 else qg23_w[j]
                st = st_ps.tile([128, 1024], F32, tag="st", name="st")
                nc.tensor.matmul(st[:, st_off:512], kt_c[0:64, ks],
                                 qg[0:64, st_off:512], start=True, stop=True)
                nc.tensor.matmul(st[:, 512 + st_off:1024], kt_c[64:128, ks],
                                 qg[64:128, st_off:512], start=True, stop=True)
                pt = pt_pool.tile([128, 1024], BF16, tag="pt", name="pt")
                st3 = st[:].rearrange("p (h c) -> p h c", c=512)
                pt3 = pt[:].rearrange("p (h c) -> p h c", c=512)
                # unshifted exp(8*st): the HW exp table has an absolute
                # error floor, so shifting all logits down corrupts rows
                # whose terms are all tiny; fp32/bf16 absorb e^75 fine.
                nc.scalar.activation(pt3[:, :, pv_off:512],
                                     st3[:, :, pv_off:512], AF.Exp,
                                     scale=8.0)
                if i >= 0:
                    # zero the upper triangle of the diagonal band: keep
                    # where local col c >= partition p
                    band = pt3[:, :, pv_off:pv_off + 128]
                    nc.gpsimd.affine_select(
                        out=band, in_=band,
                        compare_op=mybir.AluOpType.is_ge,
                        fill=0.0, base=0,
                        pattern=[[0, 2], [1, 128]],
                        channel_multiplier=-1)
                return pt, pv_off

            def do_pv(pair, k, pt, pv_off):
                if k == 0:
                    pv_tiles[pair] = (
                        pv_ps.tile([65, 512], F32, tag="pv", name="pv_a"),
                        pv_ps.tile([65, 512], F32, tag="pv", name="pv_b"))
                pv_a, pv_b = pv_tiles[pair]
                vo_c = vo_w[k // 4]
                vsl = vo_c[:, (k % 4) * 65:(k % 4) * 65 + 65]
                nc.tensor.matmul(pv_a[:, pv_off:512], vsl,
                                 pt[:, pv_off:512],
                                 start=(k == 0), stop=(k == klast))
                nc.tensor.matmul(pv_b[:, pv_off:512], vsl,
                                 pt[:, 512 + pv_off:1024],
                                 start=(k == 0), stop=(k == klast))

            def do_norm(pair):
                hid = hid01_w[j] if pair == 0 else hid23_w[j]
                pv_a, pv_b = pv_tiles.pop(pair)
                # drain pv psum to SBUF immediately (frees the banks for the
                # next pair's accumulation without waiting on the ACT queue)
                pvc = []
                for pv in (pv_a, pv_b):
                    c = rec_pool.tile([65, 512], F32, tag="pvc", name="pvc")
                    nc.vector.tensor_copy(c[:], pv[:])
                    pvc.append(c)
                for half, pv in ((0, pvc[0]), (1, pvc[1])):
                    # bit-exact DVE reciprocal: den spans ~93 e-folds
                    # (3.6e-8 .. 8.6e32), outside the accurate domain of
                    # any ACT ln/exp reciprocal chain
                    rec = rec_pool.tile([1, 512], F32, tag="rec", name="rec")
                    nc.vector.reciprocal(rec[:], pv[64:65, :])
                    recr = rec_pool.tile([64, 512], F32, tag="recr",
                                         name="recr")
                    nc.gpsimd.partition_broadcast(recr[:], rec[:])
                    nc.vector.tensor_mul(hid[half * 64:half * 64 + 64, :],
                                         pv[0:64, :], recr[:])

            for (pair, k) in jobs:
                cur = do_st(pair, k)
                if pend is not None:
                    do_pv(*pend)
                    if pend[1] == klast:  # pair-01 accumulation finished
                        do_norm(pend[0])
                        yield  # extra turn: fc/proj filler covers norm drain
                pend = (pair, k) + cur
                yield
            do_pv(*pend)
            do_norm(pend[0])

        def emit_fc(j, tail=False):
            nd = 0
            for mm2 in range(2):
                stage = fco_pool.tile([128, 2, 2048], F16, tag="fco",
                                      name="stage")
                for m2 in range(2):
                    m = 2 * mm2 + m2
                    msl = bass.ts(m, 128)
                    for eo in range(4):
                        fc_ps = misc_ps.tile([128, 512], F32, tag="mm",
                                             name="fc_ps")
                        nc.tensor.matmul(fc_ps[:], hid01_w[j][:, msl],
                                         wfc0_sb[:, bass.ts(eo, 512)],
                                         start=True, stop=False)
                        nc.tensor.matmul(fc_ps[:], hid23_w[j][:, msl],
                                         wfc1_sb[:, bass.ts(eo, 512)],
                                         start=False, stop=True)
                        dst = stage[:, m2, bass.ts(eo, 512)]
                        if tail and nd % 2 == 1:
                            # at the tail ACT is idle: alternate drains so
                            # the psum pool is never drain-paced
                            nc.scalar.activation(dst, fc_ps[:], AF.Copy)
                        else:
                            nc.vector.tensor_copy(dst, fc_ps[:])
                        nd += 1
                        yield
                nc.scalar.dma_start(
                    out[bass.ds(4 * j + 2 * mm2, 2), :, :]
                    .rearrange("c p n -> p c n"),
                    stage[:])

        def emit_consts():
            # ordered by first use: Q-bias, G, K, V, transpose, mask, FC
            for dst, srcap in ((bq_sb, bq2), (wg_sb, wg), (bg01_sb, bg01),
                               (bg23_sb, bg23), (wk_sb, wk), (bk_sb, bk2),
                               (wv_sb, wv), (bv_sb, bvv), (id_sb, ident),
                               (mask_sb, mask2)):
                nc.sync.dma_start(dst[:], srcap[:])
                yield
            nc.sync.dma_start(wfc0_sb[:], wfc[0:128, :])
            yield
            nc.sync.dma_start(wfc1_sb[:], wfc[128:256, :])
            yield

        def emit_warm(n):
            # junk matmuls: keep the PE busy during the DMA-bound ramp so
            # the HAM clock gate reaches full speed before real work peaks
            for _ in range(n):
                jt = pv_ps.tile([65, 512], F32, tag="pv", name="warm")
                nc.tensor.matmul(jt[:], wq_sb[:, 0:65], wq_sb[:, 0:512],
                                 start=True, stop=True)
                yield

        from itertools import chain as ichain

        def drain(g):
            for _ in g:
                pass

        def rr(pairs):
            """round-robin emission: [(generator, steps_per_turn)]"""
            live = [[g, w] for g, w in pairs]
            while live:
                for gw in list(live):
                    g, w = gw
                    try:
                        for _ in range(w):
                            next(g)
                    except StopIteration:
                        live.remove(gw)

        # Phase A: window-0/1 loads + consts + proj(0), PE kept warm
        rr([(ichain(emit_loads(0), emit_loads(1)), 4),
            (emit_consts(), 3), (emit_proj(0), 1), (emit_warm(60), 4)])
        # Phase B: attn(0) with proj(1) + loads(2)
        rr([(emit_attn(0), 1), (emit_loads(2), 2), (emit_proj(1), 1)])
        # Phase C: attn(1) + proj(2) + loads(3) + fc(0)
        rr([(emit_attn(1), 1), (emit_loads(3), 2), (emit_proj(2), 1),
            (emit_fc(0), 1)])
        # Phase D: attn(2) + proj(3) + fc(1)
        rr([(emit_attn(2), 1), (emit_proj(3), 1), (emit_fc(1), 1)])
        # Phase E: attn(3) + fc(2)
        rr([(emit_attn(3), 1), (emit_fc(2), 1)])
        drain(emit_fc(3, tail=True))


def shard_inputs(inputs):
    """full inputs -> list of 8 per-core in_maps (numpy, device layouts)"""
    f16 = np.float16
    f32 = np.float32
    bf16 = None
    import ml_dtypes
    bf16 = ml_dtypes.bfloat16
    q = np.asarray(inputs["q"], f32)[0]
    k = np.asarray(inputs["k"], f32)[0]
    v = np.asarray(inputs["v"], f32)[0]
    Wq = np.asarray(inputs["Wq"], f32)
    Wk = np.asarray(inputs["Wk"], f32)
    Wv = np.asarray(inputs["Wv"], f32)
    bq = np.asarray(inputs["bq"], f32)
    bk = np.asarray(inputs["bk"], f32)
    bv = np.asarray(inputs["bv"], f32)
    WG = np.asarray(inputs["WG"], f32)
    bG = np.asarray(inputs["bG"], f32)
    Wfc = np.asarray(inputs["Wfc"], f32)

    qT = np.ascontiguousarray(q.T.astype(f16)).reshape(16, 128, N)
    kT = np.ascontiguousarray(k.T.astype(f16)).reshape(16, 128, N)
    vT = np.ascontiguousarray(v.T.astype(f16)).reshape(16, 128, N)
    ident = np.eye(64, dtype=f32)
    # mask band M[p, c] = 1 if c >= p else 0, duplicated: [M | M]
    M = (np.arange(128)[None, :] >= np.arange(128)[:, None]).astype(bf16)
    mask2 = np.concatenate([M, M], axis=1).copy()

    def chunked(w):
        # [E, M] -> [128, 16*M]: e-chunk ec at cols [M*ec, M*ec+M)
        M_ = w.shape[1]
        return np.ascontiguousarray(
            w.reshape(16, 128, M_).transpose(1, 0, 2).reshape(128, 16 * M_))

    maps = []
    for h in range(HK):
        sl = slice(h * D, (h + 1) * D)
        wq_h = Wq[:, sl]
        wk_h = Wk[:, sl]
        wv_h = Wv[:, sl]
        m = {
            "qT": qT, "kT": kT, "vT": vT,
            "wq": chunked(np.concatenate([wq_h, wq_h], 1)).astype(f16),
            "wk": chunked(np.concatenate([wk_h, wk_h], 1)).astype(f16),
            "wv": chunked(wv_h).astype(f16),
            "bq2": np.concatenate([bq[sl], bq[sl]]).reshape(128, 1).copy(),
            "bk2": np.concatenate([bk[sl], bk[sl]]).reshape(128, 1).copy(),
            "bvv": bv[sl].reshape(64, 1).copy(),
            "wg": np.concatenate([WG[h], WG[h]], 0).copy(),  # [128, 256]
            "bg01": bG[h, 0:128].reshape(128, 1).copy(),
            "bg23": bG[h, 128:256].reshape(128, 1).copy(),
            "wfc": Wfc[h * 256:(h + 1) * 256, :].copy(),
            "ident": ident,
            "mask2": mask2,
        }
        maps.append(m)
    return maps


_compiled = None
last_results = None


def get_compiled():
    global _compiled
    if _compiled is None:
        _compiled = build_program()
    return _compiled


def kernel(**inputs):
    global last_results
    nc = get_compiled()
    in_maps = shard_inputs(inputs)
    last_results = bass_utils.run_bass_kernel_spmd(
        nc, in_maps, core_ids=list(range(8)))
    bfc = np.asarray(inputs["bfc"], np.float32)
    acc = np.zeros((N, E), np.float32)
    for res in last_results.results:
        acc += res["out"].reshape(N, E).astype(np.float32)
    full = acc + bfc[None, :]
    return full.reshape(1, N, E)
